# revision 38
# baseline (speedup 1.0000x reference)
"""Trainium2 Bass kernel for nn_GATModel (3-layer per-edge-head-attention GNN).

Strategy:
- Sort edges by source node; shard by source-node range across 8 cores
  (2500 nodes/core). Each core computes the message rows for its own nodes
  only, so no AllReduce is needed on messages -- just an AllGather of the
  hidden state each layer (K/V need all nodes).
- Hidden state lives in transposed layout h_T [256, nodes] (feature on
  partition) so all projections are PE matmuls with weights stationary.
- K/V are materialized row-major (node-major) in a fused KV table
  [padded_nodes, 512] for single indirect-DMA gather per edge tile.
- Per-edge attention runs on the Vector engine in "layout A" (edge on
  partition): broadcast-AP multiplies + grouped reduces compute the 4x4
  head-pair scores and softmax; the weighted-V product is folded into the
  segment-sum via PE matmuls against a 0/1 selection matrix (built with
  iota==src compares), accumulated in PSUM per 128-source-node window.

Wire-traffic minimization (the axon tunnel runs at ~45 MB/s, so host<->device
bytes dominate wall time):
- x ships once, fp16, transposed; layer-0 hidden is computed for own nodes
  only and AllGathered on-device (no replicated full-graph input).
- All replicated weights/constants are packed into one flat f32 blob that is
  sharded across the 8 cores and AllGathered on-device (4MB on the wire
  instead of 32MB).
- The kernel returns only the fp16 FFN adjustment in transposed layout; the
  final residual add (x + adjustment) happens on the host in f32.
"""
import zlib
import numpy as np

import concourse.bass as bass
import concourse.bacc as bacc
import concourse.mybir as mybir
import concourse.tile as tile
from concourse.bass_utils import run_bass_kernel_spmd

FP = mybir.dt.float32
F16 = mybir.dt.float16
I32 = mybir.dt.int32
I16 = mybir.dt.int16
I8 = mybir.dt.int8
AX = mybir.AxisListType
ALU = mybir.AluOpType
ACTF = mybir.ActivationFunctionType

N_CORES = 8
P = 128
HID = 256
N_NODES = 20000
NPC = N_NODES // N_CORES      # 2500 real nodes per core
NWIN = (NPC + P - 1) // P     # 20 windows
NPCP = NWIN * P               # 2560 padded nodes per core
NPAD = N_CORES * NPCP         # 20480 padded global nodes
N_LAYERS = 3
EPS = 1e-5

_CACHE = {}
_PREP_CACHE = {}
_RUNNER_CACHE = {}


def _get_runner(nc):
    """Per-call `run_bass_kernel_spmd` re-traces and re-lowers the whole BIR
    (jax.jit(shard_map(...)) is rebuilt every invocation). Build the identical
    jitted wrapper once and reuse it; also create the donated zero output
    buffers on-device instead of shipping them over the (slow) axon tunnel."""
    key = id(nc)
    if key in _RUNNER_CACHE:
        return _RUNNER_CACHE[key]
    import jax
    import jax.numpy as jnp
    from concourse import bass2jax as b2j

    b2j.install_neuronx_cc_hook()
    assert not (nc.dbg_addr is not None and nc.dbg_callbacks)
    partition_name = nc.partition_id_tensor.name if nc.partition_id_tensor else None
    dbg_name = nc.dbg_addr.name if nc.dbg_addr is not None else None

    in_names, out_names, out_avals, zero_shapes = [], [], [], []
    for alloc in nc.m.functions[0].allocations:
        if not isinstance(alloc, mybir.MemoryLocationSet):
            continue
        name = alloc.memorylocations[0].name
        if alloc.kind == "ExternalInput":
            if name != partition_name:
                in_names.append(name)
        elif alloc.kind == "ExternalOutput":
            shape = tuple(alloc.tensor_shape)
            dtype = mybir.dt.np(alloc.dtype)
            out_names.append(name)
            out_avals.append(jax.core.ShapedArray(shape, dtype))
            zero_shapes.append(((N_CORES * shape[0],) + shape[1:], dtype))
    n_params = len(in_names)
    n_outs = len(out_avals)
    all_names = list(in_names) + list(out_names)
    if partition_name is not None:
        all_names.append(partition_name)

    def _body(*args):
        operands = list(args)
        if partition_name is not None:
            operands.append(b2j.partition_id_tensor())
        outs = b2j._bass_exec_p.bind(
            *operands,
            out_avals=tuple(out_avals),
            in_names=tuple(all_names),
            out_names=tuple(out_names),
            lowering_input_output_aliases=(),
            sim_require_finite=True,
            sim_require_nnan=True,
            nc=nc,
        )
        return tuple(outs)

    devices = jax.devices()[:N_CORES]
    mesh = b2j.Mesh(np.asarray(devices), ("core",))
    in_specs = (b2j.PartitionSpec("core"),) * (n_params + n_outs)
    out_specs = (b2j.PartitionSpec("core"),) * n_outs
    donate = tuple(range(n_params, n_params + n_outs))
    sharded = jax.jit(
        b2j.shard_map(_body, mesh=mesh, in_specs=in_specs,
                      out_specs=out_specs, check_rep=False),
        donate_argnums=donate, keep_unused=True,
    )
    sharding = jax.sharding.NamedSharding(mesh, b2j.PartitionSpec("core"))
    make_zeros = jax.jit(
        lambda: tuple(jnp.zeros(s, d) for s, d in zero_shapes),
        out_shardings=tuple(sharding for _ in zero_shapes),
    )

    state = {"z": None}

    def run(in_maps):
        def core_in(m, name):
            if name == dbg_name and name not in m:
                return np.zeros((1, 2), np.uint32)
            return np.asarray(m[name])

        concat_in = [
            np.concatenate([core_in(in_maps[c], name) for c in range(N_CORES)], axis=0)
            for name in in_names
        ]
        zeros = state["z"] if state["z"] is not None else make_zeros()
        out_arrs = sharded(*concat_in, *zeros)
        # prefetch donated output buffers for the next call (async, overlaps
        # with the result download below)
        state["z"] = make_zeros()

        # every core's output already holds the AllGathered result of all
        # cores, so fetch only core 0's shard (one tunnel RPC)
        def shard0(arr):
            for s in arr.addressable_shards:
                if all(idx.start in (0, None) for idx in s.index):
                    return np.asarray(s.data)
            return np.asarray(arr).reshape(N_CORES, *arr.shape[1:])[0]

        return {name: shard0(out_arrs[i]) for i, name in enumerate(out_names)}

    _RUNNER_CACHE[key] = run
    return run


def _blob_layout(NTIL):
    specs = [
        ("WinT", (P, 2, HID)),
        ("WqT3", (P, 2, N_LAYERS, HID)),
        ("WkvT3", (P, 2, N_LAYERS, 2 * HID)),
        ("WoT3", (P, 2, N_LAYERS, HID)),
        ("W1T", (P, 2, HID)),
        ("W2T", (P, 2, HID)),
        ("bias_pk", (P, 6)),
        ("lng_pk", (P, 2 * N_LAYERS)),
        ("lnb_pk", (P, 2 * N_LAYERS)),
        ("bo_pk", (P, 2 * N_LAYERS)),
        ("bkv", (1, N_LAYERS, 2 * HID)),
        ("bq", (1, N_LAYERS, HID)),
        ("Eemb", (3, N_LAYERS, HID)),
        ("iota", (P, P)),
        ("sx_pk", (P, 2)),
        ("wbase", (1, NTIL)),
    ]
    off = {}
    o = 0
    for n, s in specs:
        off[n] = (o, s)
        o += int(np.prod(s))
    # chunk (tot/8) must be a multiple of 128 so the DRAM->DRAM bounce can be
    # expressed as a 2-D DMA whose dims fit 16-bit descriptor fields
    pad = (-o) % (N_CORES * P)
    return off, o + pad


def _mega_layout(NTIL):
    """All per-core uploads packed into ONE int8 tensor (each extra jit arg
    costs ~14ms of axon transfer overhead). Byte offsets, all 4-aligned."""
    _, blob_tot = _blob_layout(NTIL)
    chunk = blob_tot // N_CORES
    secs = {}
    o = 0
    for name, nbytes in [
        ("x8", HID * NPCP),
        ("TC", 3 * NPCP * 4),
        ("wchunk", chunk * 2),
        ("tgt16", P * NTIL * 2),
        ("srcl8", P * NTIL),
    ]:
        assert o % 4 == 0
        secs[name] = (o, nbytes)
        o += nbytes + ((-nbytes) % 4)
    return secs, o


def _prep_edges(edges, edge_types):
    src = np.asarray(edges[:, 0], dtype=np.int64)
    tgt = np.asarray(edges[:, 1], dtype=np.int64)
    et = np.asarray(edge_types, dtype=np.int64)
    order = np.argsort(src, kind="stable")
    src_s, tgt_s, et_s = src[order], tgt[order], et[order]
    core_of = src_s // NPC
    local = src_s - core_of * NPC
    win = local // P
    srcl = local - win * P

    cnt = np.zeros((N_CORES, NWIN), dtype=np.int64)
    np.add.at(cnt, (core_of, win), 1)
    T_w = np.maximum(1, -(-cnt.max(axis=0) // P)).astype(np.int64)
    NTIL = int(T_w.sum())
    tbase = np.concatenate([[0], np.cumsum(T_w)])

    sched = []
    for w in range(NWIN):
        for k in range(int(T_w[w])):
            sched.append((w, k == 0, k == int(T_w[w]) - 1))

    tgt_ix = np.zeros((N_CORES, P, NTIL), np.int16)
    src_ix = np.zeros((N_CORES, P, NTIL), np.int16)
    srcl_f = np.full((N_CORES, P, NTIL), -1, np.int8)
    TC = np.zeros((N_CORES, 3, NPCP), np.float32)
    np.add.at(TC, (core_of, et_s, local), 1.0)

    for c in range(N_CORES):
        m = core_of == c
        tw, sw, lw = tgt_s[m], srcl[m], win[m]
        for w in range(NWIN):
            wm = lw == w
            k = int(wm.sum())
            if k == 0:
                continue
            idx = np.arange(k)
            cols = (tbase[w] + idx // P).astype(np.int64)
            rows = idx % P
            tg = tw[wm]
            tgt_ix[c, rows, cols] = ((tg // NPC) * NPCP + (tg % NPC)).astype(np.int16)
            src_ix[c, rows, cols] = (w * P + sw[wm]).astype(np.int16)
            srcl_f[c, rows, cols] = sw[wm].astype(np.int8)
    return NTIL, sched, tgt_ix, src_ix, srcl_f, TC


def _prep_edges_cached(edges, edge_types):
    e = np.ascontiguousarray(edges)
    t = np.ascontiguousarray(edge_types)
    key = (zlib.crc32(e.tobytes()), zlib.crc32(t.tobytes()), e.shape, t.shape)
    if key not in _PREP_CACHE:
        _PREP_CACHE[key] = _prep_edges(e, t)
    return _PREP_CACHE[key]


def _build_program(NTIL, sched, qkv_bias):
    nc = bacc.Bacc("TRN2", target_bir_lowering=False, debug=False,
                   enable_asserts=True, num_devices=N_CORES)

    def inp(name, shape, dt=FP):
        return nc.dram_tensor(name, shape, dt, kind="ExternalInput").ap()

    _BLOB_OFF, _BLOB_TOT = _blob_layout(NTIL)
    _BLOB_CHUNK = _BLOB_TOT // N_CORES
    MEGA, MEGA_NB = _mega_layout(NTIL)

    mega = inp("mega", [MEGA_NB], I8)

    def sec(name, dt, shape):
        o, nb = MEGA[name]
        v = mega[o:o + nb]
        if dt != I8:
            v = v.bitcast(dt)
        pat = " ".join(f"d{i}" for i in range(len(shape)))
        kw = {f"d{i}": shape[i] for i in range(len(shape) - 1)}
        return v.rearrange(f"({pat}) -> {pat}", **kw)

    x_ownT8 = sec("x8", I8, (HID, NPCP))
    wchunk = sec("wchunk", F16, (P, _BLOB_CHUNK // P))
    tgt_i = sec("tgt16", I16, (P, NTIL))
    srcl_i = sec("srcl8", I8, (P, NTIL))
    TC_i = sec("TC", FP, (3, NPCP))

    # int8 adjustment in transposed layout; the last 4 columns carry the
    # per-feature f32 dequant scale bit-packed as 4 int8 bytes. All cores'
    # results are AllGathered on-device so the host fetches ONE core's shard
    # (a single tunnel RPC instead of 8).
    out_all = nc.dram_tensor("out_all", [N_CORES, HID, NPCP + 4], I8,
                             kind="ExternalOutput").ap()
    out_i8 = nc.dram_tensor("o_bounce", [HID, NPCP + 4], I8).ap()
    out_gath = nc.dram_tensor("out_gath", [N_CORES, HID, NPCP + 4], I8,
                              addr_space="Shared").ap()

    wblob = nc.dram_tensor("wblob", [_BLOB_TOT], F16, addr_space="Shared").ap()
    wbounce = nc.dram_tensor("wbounce", [_BLOB_CHUNK], F16).ap()
    hT_full = nc.dram_tensor("hT_full", [N_CORES, HID, NPCP], FP,
                             addr_space="Shared").ap()
    h_bounce = nc.dram_tensor("h_bounce", [HID, NPCP], FP).ap()
    KVtab = nc.dram_tensor("KVtab", [NPAD, 2 * HID], FP).ap()
    Qtab = nc.dram_tensor("Qtab", [NPCP, HID], FP).ap()

    NCT = NPCP // 512  # 5 column tiles of own nodes

    with tile.TileContext(nc) as tc:
        with (
            tc.tile_pool(name="wts", bufs=1) as wc,
            tc.tile_pool(name="state", bufs=1) as stpool,
        ):
            # gather the replicated weight/constant blob from all cores.
            # (the collective input needs an in-program writer for the tile
            # scheduler to order it, so bounce the input chunk through an
            # internal DRAM tensor first)
            nc.sync.dma_start(wbounce[:].rearrange("(p n) -> p n", p=P), wchunk)
            nc.gpsimd.collective_compute(
                "AllGather", ALU.bypass,
                replica_groups=[list(range(N_CORES))],
                ins=[wbounce[:]],
                outs=[wblob[:]],
            )

            with tc.tile_pool(name="wstg", bufs=1) as wstg:
                def load_w(name):
                    o, s = _BLOB_OFF[name]
                    n = int(np.prod(s))
                    st = wstg.tile(list(s), F16, tag=f"s_{name}")
                    pat = " ".join(f"d{i}" for i in range(len(s)))
                    kw = {f"d{i}": s[i] for i in range(len(s) - 1)}
                    nc.sync.dma_start(st[:], wblob[o:o + n].rearrange(f"({pat}) -> {pat}", **kw))
                    t = wc.tile(list(s), FP, tag=f"w_{name}")
                    nc.scalar.copy(out=t[:], in_=st[:])
                    return t

                WinT_s = load_w("WinT")
                WqT_s = load_w("WqT3")
                WkvT_s = load_w("WkvT3")
                WoT_s = load_w("WoT3")
                W1T_s = load_w("W1T")
                W2T_s = load_w("W2T")
                bias_s = load_w("bias_pk")
                lng_s = load_w("lng_pk")
                lnb_s = load_w("lnb_pk")
                bo_s = load_w("bo_pk")
                Eemb_s = load_w("Eemb")
                iota_s = load_w("iota")
                sx_s = load_w("sx_pk")
                if qkv_bias:
                    bkv_s = load_w("bkv")
                    bq_s = load_w("bq")
                wbase_s = load_w("wbase")
                ones_col = wc.tile([P, 1], FP)
                nc.vector.memset(ones_col[:], 1.0)
                ones_row = wc.tile([1, P], FP)
                nc.vector.memset(ones_row[:], 1.0)
                # edge index uploads are int16/int8; widen on-device.
                # src is derived: src = window_base(tile) + max(srcl, 0)
                tgt16 = wstg.tile([P, NTIL], I16, tag="tgt16")
                nc.sync.dma_start(tgt16[:], tgt_i)
                tgt_s = wc.tile([P, NTIL], I32)
                nc.vector.tensor_copy(out=tgt_s[:], in_=tgt16[:])
                srcl8 = wstg.tile([P, NTIL], I8, tag="srcl8")
                nc.sync.dma_start(srcl8[:], srcl_i)
                srcl_s = wc.tile([P, NTIL], FP)
                nc.vector.tensor_copy(out=srcl_s[:], in_=srcl8[:])
                with tc.tile_pool(name="wps", bufs=1, space="PSUM") as wps:
                    ps_wb = wps.tile([P, NTIL], FP, tag="pswb", space="PSUM")
                    nc.tensor.matmul(out=ps_wb[:], lhsT=ones_row[:], rhs=wbase_s[:],
                                     start=True, stop=True)
                    srcf = wstg.tile([P, NTIL], FP, tag="srcf")
                    nc.vector.tensor_scalar(out=srcf[:], in0=srcl_s[:], scalar1=0.0,
                                            scalar2=None, op0=ALU.max)
                    nc.vector.tensor_tensor(out=srcf[:], in0=srcf[:], in1=ps_wb[:], op=ALU.add)
                    src_s = wc.tile([P, NTIL], I32)
                    nc.vector.tensor_copy(out=src_s[:], in_=srcf[:])
            TC_s = wc.tile([3, NPCP], FP)
            nc.sync.dma_start(TC_s[:], TC_i)

            Msb = stpool.tile([P, 2, NPCP], FP)
            z_sb = stpool.tile([P, 2, NPCP], FP)
            hn_sb = stpool.tile([P, 2, NPCP], FP)
            f1_sb = z_sb

            # ---------------- in-proj (own nodes only) ----------------
            with (
                tc.tile_pool(name="mmp", bufs=3) as mp,
                tc.tile_pool(name="mps", bufs=2, space="PSUM") as pp,
            ):
                for ct in range(NCT):
                    cs = slice(ct * 512, (ct + 1) * 512)
                    xh = mp.tile([P, 2, 512], I8, tag="xh")
                    nc.sync.dma_start(xh[:], x_ownT8[:, cs].rearrange("(c x) n -> x c n", c=2))
                    xf = mp.tile([P, 2, 512], FP, tag="xf")
                    for ic in range(2):
                        nc.vector.tensor_scalar(out=xf[:, ic, :], in0=xh[:, ic, :],
                                                scalar1=sx_s[:, ic:ic + 1],
                                                scalar2=None, op0=ALU.mult)
                    for oc in range(2):
                        ps = pp.tile([P, 512], FP, tag="pin", space="PSUM")
                        for ic in range(2):
                            nc.tensor.matmul(
                                out=ps[:],
                                lhsT=WinT_s[:, ic, oc * P:(oc + 1) * P],
                                rhs=xf[:, ic, :],
                                start=(ic == 0), stop=(ic == 1),
                            )
                        nc.scalar.copy(out=hn_sb[:, oc, cs], in_=ps[:])
                        nc.vector.tensor_scalar(out=hn_sb[:, oc, cs], in0=hn_sb[:, oc, cs],
                                                scalar1=bias_s[:, oc:oc + 1],
                                                scalar2=None, op0=ALU.add)

            # ---------------- layers ----------------
            for l in range(N_LAYERS):
                # gather full hidden state (needed for K/V of all nodes)
                with tc.tile_pool(name="agp", bufs=1) as mp:
                    hb = mp.tile([P, 2, NPCP], FP, tag="hb")
                    nc.vector.tensor_copy(out=hb[:], in_=hn_sb[:])
                    nc.sync.dma_start(h_bounce[:].rearrange("(c x) n -> x c n", c=2), hb[:])
                    nc.gpsimd.collective_compute(
                        "AllGather", ALU.bypass,
                        replica_groups=[list(range(N_CORES))],
                        ins=[h_bounce[:]],
                        outs=[hT_full[:]],
                    )

                # ---- K/V table (all nodes) + Q table (own) ----
                with (
                    tc.tile_pool(name="kvp", bufs=4) as mp,
                    tc.tile_pool(name="kvps", bufs=3, space="PSUM") as pp,
                ):
                    for ch in range(NPAD // P):
                        blk, off = divmod(ch * P, NPCP)
                        ns = slice(ch * P, (ch + 1) * P)
                        hc = mp.tile([P, 2, P], FP, tag="hc")
                        nc.sync.dma_start(hc[:], hT_full[blk][:, off:off + P].rearrange("(c x) n -> x c n", c=2))
                        ps = pp.tile([P, 2 * HID], FP, tag="pkv", space="PSUM")
                        for ic in range(2):
                            nc.tensor.matmul(
                                out=ps[:], lhsT=hc[:, ic, :],
                                rhs=WkvT_s[:, ic, l, :],
                                start=(ic == 0), stop=((not qkv_bias) and ic == 1),
                            )
                        if qkv_bias:
                            nc.tensor.matmul(out=ps[:], lhsT=ones_row[:],
                                             rhs=bkv_s[:, l, :], start=False, stop=True)
                        kv = mp.tile([P, 2 * HID], FP, tag="kv")
                        nc.scalar.copy(out=kv[:], in_=ps[:])
                        nc.sync.dma_start(KVtab[ns, :], kv[:])
                    for ch in range(NPCP // P):
                        ns = slice(ch * P, (ch + 1) * P)
                        ps = pp.tile([P, HID], FP, tag="pq", space="PSUM")
                        for ic in range(2):
                            nc.tensor.matmul(
                                out=ps[:], lhsT=hn_sb[:, ic, ns],
                                rhs=WqT_s[:, ic, l, :],
                                start=(ic == 0), stop=((not qkv_bias) and ic == 1),
                            )
                        if qkv_bias:
                            nc.tensor.matmul(out=ps[:], lhsT=ones_row[:],
                                             rhs=bq_s[:, l, :], start=False, stop=True)
                        q = mp.tile([P, HID], FP, tag="q")
                        nc.scalar.copy(out=q[:], in_=ps[:])
                        nc.sync.dma_start(Qtab[ns, :], q[:])

                # ---- edge loop ----
                with (
                    tc.tile_pool(name="gath", bufs=4) as gp,
                    tc.tile_pool(name="work", bufs=2) as wp,
                    tc.tile_pool(name="small", bufs=4) as sp,
                    tc.tile_pool(name="eps", bufs=2, space="PSUM") as pp,
                ):
                    ps0 = ps1 = None
                    for t, (w, first, last) in enumerate(sched):
                        if first:
                            ps0 = pp.tile([P, P], FP, tag="ps0", space="PSUM")
                            ps1 = pp.tile([P, P], FP, tag="ps1", space="PSUM")
                        kvg = gp.tile([P, 2 * HID], FP, tag="kvg")
                        nc.gpsimd.indirect_dma_start(
                            out=kvg[:], out_offset=None, in_=KVtab[:],
                            in_offset=bass.IndirectOffsetOnAxis(ap=tgt_s[:, t:t + 1], axis=0))
                        qg = gp.tile([P, HID], FP, tag="qg")
                        nc.gpsimd.indirect_dma_start(
                            out=qg[:], out_offset=None, in_=Qtab[:],
                            in_offset=bass.IndirectOffsetOnAxis(ap=src_s[:, t:t + 1], axis=0))
                        Kv = kvg[:, 0:HID]
                        Vv = kvg[:, HID:2 * HID]

                        Pt = wp.tile([P, 4, 4, 64], FP, tag="Pt")
                        nc.vector.tensor_tensor(
                            out=Pt[:],
                            in0=qg[:].rearrange("p (h d) -> p h d", h=4).unsqueeze(2).broadcast_to([P, 4, 4, 64]),
                            in1=Kv.rearrange("p (g d) -> p g d", g=4).unsqueeze(1).broadcast_to([P, 4, 4, 64]),
                            op=ALU.mult)
                        S = sp.tile([P, 16], FP, tag="S")
                        nc.vector.reduce_sum(out=S[:], in_=Pt[:].rearrange("p h g d -> p (h g) d"), axis=AX.X)
                        E = sp.tile([P, 16], FP, tag="E")
                        nc.scalar.activation(out=E[:], in_=S[:], func=ACTF.Exp, scale=0.125)
                        D = sp.tile([P, 4], FP, tag="D")
                        nc.vector.reduce_sum(out=D[:], in_=E[:].rearrange("p (h g) -> p h g", h=4), axis=AX.X)
                        R = sp.tile([P, 4], FP, tag="R")
                        nc.vector.reciprocal(out=R[:], in_=D[:])
                        Wt = sp.tile([P, 4, 4], FP, tag="Wt")
                        nc.vector.tensor_tensor(out=Wt[:], in0=E[:].rearrange("p (h g) -> p h g", h=4),
                                                in1=R[:].unsqueeze(2).broadcast_to([P, 4, 4]), op=ALU.mult)
                        P2 = wp.tile([P, 4, 64, 4], FP, tag="P2")
                        nc.vector.tensor_tensor(
                            out=P2[:],
                            in0=Wt[:].unsqueeze(2).broadcast_to([P, 4, 64, 4]),
                            in1=Vv.rearrange("p (g d) -> p d g", g=4).unsqueeze(1).broadcast_to([P, 4, 64, 4]),
                            op=ALU.mult)
                        Seg = wp.tile([P, P], FP, tag="Seg")
                        nc.vector.tensor_scalar(out=Seg[:], in0=iota_s[:], scalar1=srcl_s[:, t:t + 1],
                                                scalar2=None, op0=ALU.is_equal)
                        for hc_i in range(2):
                            ps = ps0 if hc_i == 0 else ps1
                            for g in range(4):
                                nc.tensor.matmul(
                                    out=ps[:],
                                    lhsT=P2[:, 2 * hc_i:2 * hc_i + 2, :, g].rearrange("p a d -> p (a d)"),
                                    rhs=Seg[:],
                                    start=(first and g == 0), stop=False,
                                )
                        if last:
                            for hc_i in range(2):
                                ps = ps0 if hc_i == 0 else ps1
                                nc.tensor.matmul(
                                    out=ps[:],
                                    lhsT=Eemb_s[:, l, hc_i * P:(hc_i + 1) * P],
                                    rhs=TC_s[:, w * P:(w + 1) * P],
                                    start=False, stop=True,
                                )
                                nc.scalar.copy(out=Msb[:, hc_i, w * P:(w + 1) * P], in_=ps[:])

                # ---- Wo-proj + residual + LN + relu (own nodes) ----
                with (
                    tc.tile_pool(name="upd", bufs=3) as mp,
                    tc.tile_pool(name="upps", bufs=2, space="PSUM") as pp,
                    tc.tile_pool(name="upst", bufs=1, space="PSUM") as pp_st,
                    tc.tile_pool(name="upbc", bufs=1, space="PSUM") as pp_bc,
                ):
                    for ct in range(NCT):
                        cs = slice(ct * 512, (ct + 1) * 512)
                        for oc in range(2):
                            ps = pp.tile([P, 512], FP, tag="pm2", space="PSUM")
                            for ic in range(2):
                                nc.tensor.matmul(
                                    out=ps[:],
                                    lhsT=WoT_s[:, ic, l, oc * P:(oc + 1) * P],
                                    rhs=Msb[:, ic, cs],
                                    start=(ic == 0), stop=(ic == 1),
                                )
                            nc.vector.tensor_tensor(out=z_sb[:, oc, cs], in0=ps[:],
                                                    in1=hn_sb[:, oc, cs], op=ALU.add)
                            nc.vector.tensor_scalar(out=z_sb[:, oc, cs], in0=z_sb[:, oc, cs],
                                                    scalar1=bo_s[:, 2 * l + oc:2 * l + oc + 1],
                                                    scalar2=None, op0=ALU.add)
                        # stats over feature dim via ones-matmul
                        ps_sum = pp_st.tile([1, 512], FP, tag="pssum", space="PSUM")
                        ps_sq = pp_st.tile([1, 512], FP, tag="pssq", space="PSUM")
                        sq = mp.tile([P, 2, 512], FP, tag="sq")
                        for oc in range(2):
                            nc.scalar.activation(out=sq[:, oc, :], in_=z_sb[:, oc, cs], func=ACTF.Square)
                        for oc in range(2):
                            nc.tensor.matmul(out=ps_sum[:], lhsT=ones_col[:], rhs=z_sb[:, oc, cs],
                                             start=(oc == 0), stop=(oc == 1))
                        for oc in range(2):
                            nc.tensor.matmul(out=ps_sq[:], lhsT=ones_col[:], rhs=sq[:, oc, :],
                                             start=(oc == 0), stop=(oc == 1))
                        mu = mp.tile([1, 512], FP, tag="mu")
                        nc.scalar.activation(out=mu[:], in_=ps_sum[:], func=ACTF.Copy, scale=1.0 / HID)
                        var = mp.tile([1, 512], FP, tag="var")
                        nc.scalar.activation(out=var[:], in_=ps_sq[:], func=ACTF.Copy, scale=1.0 / HID)
                        musq = mp.tile([1, 512], FP, tag="musq")
                        nc.scalar.activation(out=musq[:], in_=mu[:], func=ACTF.Square)
                        nc.vector.tensor_tensor(out=var[:], in0=var[:], in1=musq[:], op=ALU.subtract)
                        lnv = mp.tile([1, 512], FP, tag="lnv")
                        nc.vector.tensor_scalar(out=lnv[:], in0=var[:], scalar1=float(EPS),
                                                scalar2=None, op0=ALU.add)
                        nc.scalar.activation(out=lnv[:], in_=lnv[:], func=ACTF.Ln)
                        rstd = mp.tile([1, 512], FP, tag="rstd")
                        nc.scalar.activation(out=rstd[:], in_=lnv[:], func=ACTF.Exp, scale=-0.5)
                        ps_mu = pp_bc.tile([P, 512], FP, tag="psmu", space="PSUM")
                        ps_rs = pp_bc.tile([P, 512], FP, tag="psrs", space="PSUM")
                        nc.tensor.matmul(out=ps_mu[:], lhsT=ones_row[:], rhs=mu[:], start=True, stop=True)
                        nc.tensor.matmul(out=ps_rs[:], lhsT=ones_row[:], rhs=rstd[:], start=True, stop=True)
                        for oc in range(2):
                            nc.vector.tensor_tensor(out=hn_sb[:, oc, cs], in0=z_sb[:, oc, cs],
                                                    in1=ps_mu[:], op=ALU.subtract)
                            nc.vector.tensor_tensor(out=hn_sb[:, oc, cs], in0=hn_sb[:, oc, cs],
                                                    in1=ps_rs[:], op=ALU.mult)
                            nc.vector.tensor_scalar(out=hn_sb[:, oc, cs], in0=hn_sb[:, oc, cs],
                                                    scalar1=lng_s[:, 2 * l + oc:2 * l + oc + 1],
                                                    scalar2=lnb_s[:, 2 * l + oc:2 * l + oc + 1],
                                                    op0=ALU.mult, op1=ALU.add)
                            nc.scalar.activation(out=hn_sb[:, oc, cs], in_=hn_sb[:, oc, cs], func=ACTF.Relu)

            # ---------------- FFN; adjustment out (residual added on host) ----------------
            with (
                tc.tile_pool(name="ffn", bufs=3) as mp,
                tc.tile_pool(name="ffps", bufs=2, space="PSUM") as pp,
            ):
                for ct in range(NCT):
                    cs = slice(ct * 512, (ct + 1) * 512)
                    for oc in range(2):
                        ps = pp.tile([P, 512], FP, tag="pf1", space="PSUM")
                        for ic in range(2):
                            nc.tensor.matmul(
                                out=ps[:], lhsT=W1T_s[:, ic, oc * P:(oc + 1) * P],
                                rhs=hn_sb[:, ic, cs],
                                start=(ic == 0), stop=(ic == 1),
                            )
                        nc.scalar.activation(out=f1_sb[:, oc, cs], in_=ps[:], func=ACTF.Relu,
                                             bias=bias_s[:, 2 + oc:3 + oc], scale=1.0)
                # f2 (the adjustment) lands in hn_sb (free by now), then is
                # quantized to int8 with a per-feature scale computed on-device
                for ct in range(NCT):
                    cs = slice(ct * 512, (ct + 1) * 512)
                    for oc in range(2):
                        ps = pp.tile([P, 512], FP, tag="pf2", space="PSUM")
                        for ic in range(2):
                            nc.tensor.matmul(
                                out=ps[:], lhsT=W2T_s[:, ic, oc * P:(oc + 1) * P],
                                rhs=f1_sb[:, ic, cs],
                                start=(ic == 0), stop=(ic == 1),
                            )
                        nc.scalar.activation(out=hn_sb[:, oc, cs], in_=ps[:], func=ACTF.Identity,
                                             bias=bias_s[:, 4 + oc:5 + oc], scale=1.0)
                amax = mp.tile([P, 2], FP, tag="amax")
                rec = mp.tile([P, 2], FP, tag="rec")
                scl2 = mp.tile([P, 2], FP, tag="scl2")
                q8 = mp.tile([P, 2, NPCP], I8, tag="q8")
                for oc in range(2):
                    nc.vector.reduce_max(out=amax[:, oc:oc + 1], in_=hn_sb[:, oc, :],
                                         axis=AX.X, apply_absolute_value=True)
                nc.vector.tensor_scalar(out=amax[:], in0=amax[:], scalar1=1e-20,
                                        scalar2=None, op0=ALU.max)
                nc.vector.reciprocal(out=rec[:], in_=amax[:])
                nc.vector.tensor_scalar(out=scl2[:], in0=amax[:], scalar1=1.0 / 127.0,
                                        scalar2=None, op0=ALU.mult)
                for oc in range(2):
                    nc.vector.tensor_scalar(out=q8[:, oc, :], in0=hn_sb[:, oc, :],
                                            scalar1=rec[:, oc:oc + 1], scalar2=127.0,
                                            op0=ALU.mult, op1=ALU.mult)
                nc.sync.dma_start(out_i8[:, 0:NPCP].rearrange("(c x) n -> x c n", c=2), q8[:])
                nc.sync.dma_start(
                    out_i8[:, NPCP:NPCP + 4].rearrange("(c x) n -> x c n", c=2),
                    scl2[:].bitcast(I8).rearrange("p (c b) -> p c b", c=2))
                nc.gpsimd.collective_compute(
                    "AllGather", ALU.bypass,
                    replica_groups=[list(range(N_CORES))],
                    ins=[out_i8[:]],
                    outs=[out_gath[:]],
                )
                nc.sync.dma_start(out_all[:], out_gath[:])

    nc.compile()
    return nc


def _chunk_wT(W):
    """W [O, I] or [L, O, I] -> device layout [128, 2, (L,) O] where
    arr[x, c, (l,) o] = W[(l,) o, c*128+x]."""
    W = np.asarray(W, np.float32)
    if W.ndim == 2:
        A = W.T.reshape(2, P, W.shape[0])            # [c, x, o]
        return np.ascontiguousarray(A.transpose(1, 0, 2))
    A = W.transpose(2, 0, 1).reshape(2, P, W.shape[0], W.shape[1])  # [c, x, l, o]
    return np.ascontiguousarray(A.transpose(1, 0, 2, 3))


def _pack2(*vs):
    # each v [256] -> [128, 2]; concat on cols
    cols = []
    for v in vs:
        cols.append(np.asarray(v, np.float32).reshape(2, P).T)
    return np.ascontiguousarray(np.concatenate(cols, axis=1))


def kernel(x, edges, edge_types, Win, b_in, Wq, bq, Wk, bk, Wv, bv,
           Eemb, Wo, bo, ln_g, ln_b, W1, b1, W2, b2):
    x = np.asarray(x, np.float32)
    NTIL, sched, tgt_ix, src_ix, srcl_f, TC = _prep_edges_cached(
        np.asarray(edges), np.asarray(edge_types))

    qkv_bias = bool(np.any(np.asarray(bq)) or np.any(np.asarray(bk)) or np.any(np.asarray(bv)))
    key = (NTIL, tuple(w for w, _, _ in sched), qkv_bias)
    if key not in _CACHE:
        _CACHE[key] = _build_program(NTIL, sched, qkv_bias)
    nc = _CACHE[key]

    # per-feature int8 quantization of x; the dequant scale ships in the blob
    # (quantize against the fp16-rounded scale the device will actually use)
    sx = np.maximum(np.abs(x).max(axis=0) / 127.0, 1e-20)
    sx = sx.astype(np.float16).astype(np.float32)
    y = x * (1.0 / sx)
    np.rint(y, out=y)
    np.clip(y, -127, 127, out=y)
    x8T = np.ascontiguousarray(y.astype(np.int8).T)  # [HID, N_NODES]

    parts = {
        "WinT": _chunk_wT(Win),
        "WqT3": _chunk_wT(Wq),
        "WkvT3": np.concatenate([_chunk_wT(Wk), _chunk_wT(Wv)], axis=3),
        "WoT3": _chunk_wT(Wo),
        "W1T": _chunk_wT(W1),
        "W2T": _chunk_wT(W2),
        "bias_pk": _pack2(b_in, b1, b2),
        "lng_pk": _pack2(*[np.asarray(ln_g, np.float32)[l] for l in range(N_LAYERS)]),
        "lnb_pk": _pack2(*[np.asarray(ln_b, np.float32)[l] for l in range(N_LAYERS)]),
        "bo_pk": _pack2(*[np.asarray(bo, np.float32)[l] for l in range(N_LAYERS)]),
        "bkv": np.concatenate([np.asarray(bk, np.float32), np.asarray(bv, np.float32)],
                              axis=1).reshape(1, N_LAYERS, 2 * HID),
        "bq": np.asarray(bq, np.float32).reshape(1, N_LAYERS, HID),
        "Eemb": np.ascontiguousarray(np.transpose(np.asarray(Eemb, np.float32), (1, 0, 2))),
        "iota": np.ascontiguousarray(np.broadcast_to(np.arange(P, dtype=np.float32), (P, P))),
        "sx_pk": np.ascontiguousarray(sx.reshape(2, P).T),
        "wbase": np.asarray([[float(w * P) for w, _, _ in sched]], np.float32),
    }
    BLOB_OFF, BLOB_TOT = _blob_layout(NTIL)
    BLOB_CHUNK = BLOB_TOT // N_CORES
    blob = np.zeros(BLOB_TOT, np.float16)
    for name, (o, s) in BLOB_OFF.items():
        a = parts[name]
        assert a.shape == s, (name, a.shape, s)
        blob[o:o + a.size] = a.ravel().astype(np.float16)

    MEGA, MEGA_NB = _mega_layout(NTIL)

    def fill(m, name, arr):
        o, nb = MEGA[name]
        m[o:o + nb] = arr.ravel().view(np.int8)

    in_maps = []
    for c in range(N_CORES):
        xT8 = np.zeros((HID, NPCP), np.int8)
        xT8[:, :NPC] = x8T[:, c * NPC:(c + 1) * NPC]
        m = np.zeros(MEGA_NB, np.int8)
        fill(m, "x8", xT8)
        fill(m, "TC", TC[c])
        fill(m, "wchunk", blob[c * BLOB_CHUNK:(c + 1) * BLOB_CHUNK])
        fill(m, "tgt16", tgt_ix[c])
        fill(m, "srcl8", srcl_f[c])
        in_maps.append({"mega": m})

    try:
        G = _get_runner(nc)(in_maps)["out_all"]       # [N_CORES, HID, NPCP+4]
    except Exception as e:
        import traceback
        print(f"cached runner failed ({e!r}); falling back to run_bass_kernel_spmd")
        traceback.print_exc()
        G = run_bass_kernel_spmd(nc, in_maps, list(range(N_CORES))).results[0]["out_all"]
    out = np.empty((N_NODES, HID), np.float32)
    for c in range(N_CORES):
        buf = G[c]                                    # [HID, NPCP+4] int8
        scl = buf[:, NPCP:NPCP + 4].copy().view(np.float32).ravel()  # [HID]
        sl = slice(c * NPC, (c + 1) * NPC)
        np.multiply(buf[:, :NPC].T, scl[None, :], out=out[sl])
        out[sl] += x[sl]
    return out


# revision 44
# speedup vs baseline: 1.0772x; 1.0772x over previous
"""Trainium2 Bass kernel for nn_GATModel (3-layer per-edge-head-attention GNN).

Strategy:
- Sort edges by source node; shard by source-node range across 8 cores
  (2500 nodes/core). Each core computes the message rows for its own nodes
  only, so no AllReduce is needed on messages -- just an AllGather of the
  hidden state each layer (K/V need all nodes).
- Hidden state lives in transposed layout h_T [256, nodes] (feature on
  partition) so all projections are PE matmuls with weights stationary.
- K/V are materialized row-major (node-major) in a fused KV table
  [padded_nodes, 512] for single indirect-DMA gather per edge tile.
- Per-edge attention runs on the Vector engine in "layout A" (edge on
  partition): broadcast-AP multiplies + grouped reduces compute the 4x4
  head-pair scores and softmax; the weighted-V product is folded into the
  segment-sum via PE matmuls against a 0/1 selection matrix (built with
  iota==src compares), accumulated in PSUM per 128-source-node window.

Wire-traffic minimization (the axon tunnel runs at ~45 MB/s, so host<->device
bytes dominate wall time):
- x ships once, fp16, transposed; layer-0 hidden is computed for own nodes
  only and AllGathered on-device (no replicated full-graph input).
- All replicated weights/constants are packed into one flat f32 blob that is
  sharded across the 8 cores and AllGathered on-device (4MB on the wire
  instead of 32MB).
- The kernel returns only the fp16 FFN adjustment in transposed layout; the
  final residual add (x + adjustment) happens on the host in f32.
"""
import zlib
import numpy as np

import concourse.bass as bass
import concourse.bacc as bacc
import concourse.mybir as mybir
import concourse.tile as tile
from concourse.bass_utils import run_bass_kernel_spmd

FP = mybir.dt.float32
F16 = mybir.dt.float16
I32 = mybir.dt.int32
I16 = mybir.dt.int16
I8 = mybir.dt.int8
AX = mybir.AxisListType
ALU = mybir.AluOpType
ACTF = mybir.ActivationFunctionType

N_CORES = 8
P = 128
HID = 256
N_NODES = 20000
NPC = N_NODES // N_CORES      # 2500 real nodes per core
NWIN = (NPC + P - 1) // P     # 20 windows
NPCP = NWIN * P               # 2560 padded nodes per core
NPAD = N_CORES * NPCP         # 20480 padded global nodes
N_LAYERS = 3
EPS = 1e-5

_CACHE = {}
_PREP_CACHE = {}
_RUNNER_CACHE = {}


def _get_runner(nc):
    """Per-call `run_bass_kernel_spmd` re-traces and re-lowers the whole BIR
    (jax.jit(shard_map(...)) is rebuilt every invocation). Build the identical
    jitted wrapper once and reuse it; also create the donated zero output
    buffers on-device instead of shipping them over the (slow) axon tunnel."""
    key = id(nc)
    if key in _RUNNER_CACHE:
        return _RUNNER_CACHE[key]
    import jax
    import jax.numpy as jnp
    from concourse import bass2jax as b2j

    b2j.install_neuronx_cc_hook()
    assert not (nc.dbg_addr is not None and nc.dbg_callbacks)
    partition_name = nc.partition_id_tensor.name if nc.partition_id_tensor else None
    dbg_name = nc.dbg_addr.name if nc.dbg_addr is not None else None

    in_names, out_names, out_avals, zero_shapes = [], [], [], []
    for alloc in nc.m.functions[0].allocations:
        if not isinstance(alloc, mybir.MemoryLocationSet):
            continue
        name = alloc.memorylocations[0].name
        if alloc.kind == "ExternalInput":
            if name != partition_name:
                in_names.append(name)
        elif alloc.kind == "ExternalOutput":
            shape = tuple(alloc.tensor_shape)
            dtype = mybir.dt.np(alloc.dtype)
            out_names.append(name)
            out_avals.append(jax.core.ShapedArray(shape, dtype))
            zero_shapes.append(((N_CORES * shape[0],) + shape[1:], dtype))
    n_params = len(in_names)
    n_outs = len(out_avals)
    all_names = list(in_names) + list(out_names)
    if partition_name is not None:
        all_names.append(partition_name)

    def _body(*args):
        operands = list(args)
        if partition_name is not None:
            operands.append(b2j.partition_id_tensor())
        outs = b2j._bass_exec_p.bind(
            *operands,
            out_avals=tuple(out_avals),
            in_names=tuple(all_names),
            out_names=tuple(out_names),
            lowering_input_output_aliases=(),
            sim_require_finite=True,
            sim_require_nnan=True,
            nc=nc,
        )
        return tuple(outs)

    devices = jax.devices()[:N_CORES]
    mesh = b2j.Mesh(np.asarray(devices), ("core",))
    in_specs = (b2j.PartitionSpec("core"),) * (n_params + n_outs)
    out_specs = (b2j.PartitionSpec("core"),) * n_outs
    donate = tuple(range(n_params, n_params + n_outs))
    sharded = jax.jit(
        b2j.shard_map(_body, mesh=mesh, in_specs=in_specs,
                      out_specs=out_specs, check_rep=False),
        donate_argnums=donate, keep_unused=True,
    )
    sharding = jax.sharding.NamedSharding(mesh, b2j.PartitionSpec("core"))
    make_zeros = jax.jit(
        lambda: tuple(jnp.zeros(s, d) for s, d in zero_shapes),
        out_shardings=tuple(sharding for _ in zero_shapes),
    )

    state = {"z": None}

    def run(in_maps):
        def core_in(m, name):
            if name == dbg_name and name not in m:
                return np.zeros((1, 2), np.uint32)
            return np.asarray(m[name])

        concat_in = [
            np.concatenate([core_in(in_maps[c], name) for c in range(N_CORES)], axis=0)
            for name in in_names
        ]
        zeros = state["z"] if state["z"] is not None else make_zeros()
        out_arrs = sharded(*concat_in, *zeros)
        # prefetch donated output buffers for the next call (async, overlaps
        # with the result download below)
        state["z"] = make_zeros()

        # every core's output already holds the AllGathered result of all
        # cores, so fetch only core 0's shard (one tunnel RPC)
        def shard0(arr):
            for s in arr.addressable_shards:
                if all(idx.start in (0, None) for idx in s.index):
                    return np.asarray(s.data)
            return np.asarray(arr).reshape(N_CORES, *arr.shape[1:])[0]

        return {name: shard0(out_arrs[i]) for i, name in enumerate(out_names)}

    _RUNNER_CACHE[key] = run
    return run


# weight matrices ship as int8 with per-input-channel scales ("<name>$s"
# entries, applied on-device after the AllGather); everything else fp16
_W8 = ("WinT", "WqT3", "WkvT3", "WoT3", "W1T", "W2T")


def _blob_layout(NTIL):
    specs = []
    for n, s in [
        ("WinT", (P, 2, HID)),
        ("WqT3", (P, 2, N_LAYERS, HID)),
        ("WkvT3", (P, 2, N_LAYERS, 2 * HID)),
        ("WoT3", (P, 2, N_LAYERS, HID)),
        ("W1T", (P, 2, HID)),
        ("W2T", (P, 2, HID)),
    ]:
        specs.append((n, s, I8))
        specs.append((n + "$s", (P, 2), F16))
    specs += [
        ("bias_pk", (P, 6), F16),
        ("lng_pk", (P, 2 * N_LAYERS), F16),
        ("lnb_pk", (P, 2 * N_LAYERS), F16),
        ("bo_pk", (P, 2 * N_LAYERS), F16),
        ("bkv", (1, N_LAYERS, 2 * HID), F16),
        ("bq", (1, N_LAYERS, HID), F16),
        ("Eemb", (3, N_LAYERS, HID), F16),
        ("iota", (P, P), F16),
        ("sx_pk", (P, 2), F16),
        ("wbase", (1, NTIL), F16),
    ]
    off = {}
    o = 0  # byte offset
    for n, s, dt in specs:
        nb = int(np.prod(s)) * (1 if dt == I8 else 2)
        off[n] = (o, s, dt)
        o += nb + ((-nb) % 4)
    # chunk (tot/8) must be a multiple of 128 so the DRAM->DRAM bounce can be
    # expressed as a 2-D DMA whose dims fit 16-bit descriptor fields
    pad = (-o) % (N_CORES * P)
    return off, o + pad


def _mega_layout(NTIL):
    """All per-core uploads packed into ONE int8 tensor (each extra jit arg
    costs ~14ms of axon transfer overhead). Byte offsets, all 4-aligned."""
    _, blob_tot = _blob_layout(NTIL)   # bytes
    chunk = blob_tot // N_CORES
    secs = {}
    o = 0
    for name, nbytes in [
        ("x8", HID * NPCP),
        ("TC", 3 * NPCP * 4),
        ("wchunk", chunk),
        ("tgt16", P * NTIL * 2),
        ("srcl8", P * NTIL),
    ]:
        assert o % 4 == 0
        secs[name] = (o, nbytes)
        o += nbytes + ((-nbytes) % 4)
    return secs, o


def _prep_edges(edges, edge_types):
    src = np.asarray(edges[:, 0], dtype=np.int64)
    tgt = np.asarray(edges[:, 1], dtype=np.int64)
    et = np.asarray(edge_types, dtype=np.int64)
    order = np.argsort(src, kind="stable")
    src_s, tgt_s, et_s = src[order], tgt[order], et[order]
    core_of = src_s // NPC
    local = src_s - core_of * NPC
    win = local // P
    srcl = local - win * P

    cnt = np.zeros((N_CORES, NWIN), dtype=np.int64)
    np.add.at(cnt, (core_of, win), 1)
    T_w = np.maximum(1, -(-cnt.max(axis=0) // P)).astype(np.int64)
    NTIL = int(T_w.sum())
    tbase = np.concatenate([[0], np.cumsum(T_w)])

    sched = []
    for w in range(NWIN):
        for k in range(int(T_w[w])):
            sched.append((w, k == 0, k == int(T_w[w]) - 1))

    tgt_ix = np.zeros((N_CORES, P, NTIL), np.int16)
    src_ix = np.zeros((N_CORES, P, NTIL), np.int16)
    srcl_f = np.full((N_CORES, P, NTIL), -1, np.int8)
    TC = np.zeros((N_CORES, 3, NPCP), np.float32)
    np.add.at(TC, (core_of, et_s, local), 1.0)

    for c in range(N_CORES):
        m = core_of == c
        tw, sw, lw = tgt_s[m], srcl[m], win[m]
        for w in range(NWIN):
            wm = lw == w
            k = int(wm.sum())
            if k == 0:
                continue
            idx = np.arange(k)
            cols = (tbase[w] + idx // P).astype(np.int64)
            rows = idx % P
            tg = tw[wm]
            tgt_ix[c, rows, cols] = ((tg // NPC) * NPCP + (tg % NPC)).astype(np.int16)
            src_ix[c, rows, cols] = (w * P + sw[wm]).astype(np.int16)
            srcl_f[c, rows, cols] = sw[wm].astype(np.int8)
    return NTIL, sched, tgt_ix, src_ix, srcl_f, TC


def _prep_edges_cached(edges, edge_types):
    e = np.ascontiguousarray(edges)
    t = np.ascontiguousarray(edge_types)
    key = (zlib.crc32(e.tobytes()), zlib.crc32(t.tobytes()), e.shape, t.shape)
    if key not in _PREP_CACHE:
        _PREP_CACHE[key] = _prep_edges(e, t)
    return _PREP_CACHE[key]


def _build_program(NTIL, sched, qkv_bias):
    nc = bacc.Bacc("TRN2", target_bir_lowering=False, debug=False,
                   enable_asserts=True, num_devices=N_CORES)

    def inp(name, shape, dt=FP):
        return nc.dram_tensor(name, shape, dt, kind="ExternalInput").ap()

    _BLOB_OFF, _BLOB_TOT = _blob_layout(NTIL)
    _BLOB_CHUNK = _BLOB_TOT // N_CORES
    MEGA, MEGA_NB = _mega_layout(NTIL)

    mega = inp("mega", [MEGA_NB], I8)

    def sec(name, dt, shape):
        o, nb = MEGA[name]
        v = mega[o:o + nb]
        if dt != I8:
            v = v.bitcast(dt)
        pat = " ".join(f"d{i}" for i in range(len(shape)))
        kw = {f"d{i}": shape[i] for i in range(len(shape) - 1)}
        return v.rearrange(f"({pat}) -> {pat}", **kw)

    x_ownT8 = sec("x8", I8, (HID, NPCP))
    wchunk = sec("wchunk", I8, (P, _BLOB_CHUNK // P))
    tgt_i = sec("tgt16", I16, (P, NTIL))
    srcl_i = sec("srcl8", I8, (P, NTIL))
    TC_i = sec("TC", FP, (3, NPCP))

    # int8 adjustment in transposed layout; the last 4 columns carry the
    # per-feature f32 dequant scale bit-packed as 4 int8 bytes. All cores'
    # results are AllGathered on-device so the host fetches ONE core's shard
    # (a single tunnel RPC instead of 8).
    out_all = nc.dram_tensor("out_all", [N_CORES, HID, NPCP + 4], I8,
                             kind="ExternalOutput").ap()
    out_i8 = nc.dram_tensor("o_bounce", [HID, NPCP + 4], I8).ap()
    out_gath = nc.dram_tensor("out_gath", [N_CORES, HID, NPCP + 4], I8,
                              addr_space="Shared").ap()

    wblob = nc.dram_tensor("wblob", [_BLOB_TOT], I8, addr_space="Shared").ap()
    wbounce = nc.dram_tensor("wbounce", [_BLOB_CHUNK], I8).ap()
    hT_full = nc.dram_tensor("hT_full", [N_CORES, HID, NPCP], FP,
                             addr_space="Shared").ap()
    h_bounce = nc.dram_tensor("h_bounce", [HID, NPCP], FP).ap()
    KVtab = nc.dram_tensor("KVtab", [NPAD, 2 * HID], FP).ap()
    Qtab = nc.dram_tensor("Qtab", [NPCP, HID], FP).ap()

    NCT = NPCP // 512  # 5 column tiles of own nodes

    with tile.TileContext(nc) as tc:
        with (
            tc.tile_pool(name="wts", bufs=1) as wc,
            tc.tile_pool(name="state", bufs=1) as stpool,
        ):
            # gather the replicated weight/constant blob from all cores.
            # (the collective input needs an in-program writer for the tile
            # scheduler to order it, so bounce the input chunk through an
            # internal DRAM tensor first)
            nc.sync.dma_start(wbounce[:].rearrange("(p n) -> p n", p=P), wchunk)
            nc.gpsimd.collective_compute(
                "AllGather", ALU.bypass,
                replica_groups=[list(range(N_CORES))],
                ins=[wbounce[:]],
                outs=[wblob[:]],
            )

            with tc.tile_pool(name="wstg", bufs=1) as wstg:
                def load_w(name):
                    o, s, dt = _BLOB_OFF[name]
                    n = int(np.prod(s))
                    st = wstg.tile(list(s), dt, tag="s_" + name.replace("$", "_"))
                    v = wblob[o:o + n * (1 if dt == I8 else 2)]
                    if dt != I8:
                        v = v.bitcast(dt)
                    pat = " ".join(f"d{i}" for i in range(len(s)))
                    kw = {f"d{i}": s[i] for i in range(len(s) - 1)}
                    nc.sync.dma_start(st[:], v.rearrange(f"({pat}) -> {pat}", **kw))
                    t = wc.tile(list(s), FP, tag="w_" + name.replace("$", "_"))
                    if dt == I8:
                        sc = load_w(name + "$s")
                        for c2 in range(2):
                            nc.vector.tensor_scalar(out=t[:, c2], in0=st[:, c2],
                                                    scalar1=sc[:, c2:c2 + 1],
                                                    scalar2=None, op0=ALU.mult)
                    else:
                        nc.scalar.copy(out=t[:], in_=st[:])
                    return t

                WinT_s = load_w("WinT")
                WqT_s = load_w("WqT3")
                WkvT_s = load_w("WkvT3")
                WoT_s = load_w("WoT3")
                W1T_s = load_w("W1T")
                W2T_s = load_w("W2T")
                bias_s = load_w("bias_pk")
                lng_s = load_w("lng_pk")
                lnb_s = load_w("lnb_pk")
                bo_s = load_w("bo_pk")
                Eemb_s = load_w("Eemb")
                iota_s = load_w("iota")
                sx_s = load_w("sx_pk")
                if qkv_bias:
                    bkv_s = load_w("bkv")
                    bq_s = load_w("bq")
                wbase_s = load_w("wbase")
                ones_col = wc.tile([P, 1], FP)
                nc.vector.memset(ones_col[:], 1.0)
                ones_row = wc.tile([1, P], FP)
                nc.vector.memset(ones_row[:], 1.0)
                # edge index uploads are int16/int8; widen on-device.
                # src is derived: src = window_base(tile) + max(srcl, 0)
                tgt16 = wstg.tile([P, NTIL], I16, tag="tgt16")
                nc.sync.dma_start(tgt16[:], tgt_i)
                tgt_s = wc.tile([P, NTIL], I32)
                nc.vector.tensor_copy(out=tgt_s[:], in_=tgt16[:])
                srcl8 = wstg.tile([P, NTIL], I8, tag="srcl8")
                nc.sync.dma_start(srcl8[:], srcl_i)
                srcl_s = wc.tile([P, NTIL], FP)
                nc.vector.tensor_copy(out=srcl_s[:], in_=srcl8[:])
                with tc.tile_pool(name="wps", bufs=1, space="PSUM") as wps:
                    ps_wb = wps.tile([P, NTIL], FP, tag="pswb", space="PSUM")
                    nc.tensor.matmul(out=ps_wb[:], lhsT=ones_row[:], rhs=wbase_s[:],
                                     start=True, stop=True)
                    srcf = wstg.tile([P, NTIL], FP, tag="srcf")
                    nc.vector.tensor_scalar(out=srcf[:], in0=srcl_s[:], scalar1=0.0,
                                            scalar2=None, op0=ALU.max)
                    nc.vector.tensor_tensor(out=srcf[:], in0=srcf[:], in1=ps_wb[:], op=ALU.add)
                    src_s = wc.tile([P, NTIL], I32)
                    nc.vector.tensor_copy(out=src_s[:], in_=srcf[:])
            TC_s = wc.tile([3, NPCP], FP)
            nc.sync.dma_start(TC_s[:], TC_i)

            Msb = stpool.tile([P, 2, NPCP], FP)
            z_sb = stpool.tile([P, 2, NPCP], FP)
            hn_sb = stpool.tile([P, 2, NPCP], FP)
            f1_sb = z_sb

            # ---------------- in-proj (own nodes only) ----------------
            with (
                tc.tile_pool(name="mmp", bufs=3) as mp,
                tc.tile_pool(name="mps", bufs=2, space="PSUM") as pp,
            ):
                for ct in range(NCT):
                    cs = slice(ct * 512, (ct + 1) * 512)
                    xh = mp.tile([P, 2, 512], I8, tag="xh")
                    nc.sync.dma_start(xh[:], x_ownT8[:, cs].rearrange("(c x) n -> x c n", c=2))
                    xf = mp.tile([P, 2, 512], FP, tag="xf")
                    for ic in range(2):
                        nc.vector.tensor_scalar(out=xf[:, ic, :], in0=xh[:, ic, :],
                                                scalar1=sx_s[:, ic:ic + 1],
                                                scalar2=None, op0=ALU.mult)
                    for oc in range(2):
                        ps = pp.tile([P, 512], FP, tag="pin", space="PSUM")
                        for ic in range(2):
                            nc.tensor.matmul(
                                out=ps[:],
                                lhsT=WinT_s[:, ic, oc * P:(oc + 1) * P],
                                rhs=xf[:, ic, :],
                                start=(ic == 0), stop=(ic == 1),
                            )
                        nc.scalar.copy(out=hn_sb[:, oc, cs], in_=ps[:])
                        nc.vector.tensor_scalar(out=hn_sb[:, oc, cs], in0=hn_sb[:, oc, cs],
                                                scalar1=bias_s[:, oc:oc + 1],
                                                scalar2=None, op0=ALU.add)

            # ---------------- layers ----------------
            for l in range(N_LAYERS):
                # gather full hidden state (needed for K/V of all nodes)
                with tc.tile_pool(name="agp", bufs=1) as mp:
                    hb = mp.tile([P, 2, NPCP], FP, tag="hb")
                    nc.vector.tensor_copy(out=hb[:], in_=hn_sb[:])
                    nc.sync.dma_start(h_bounce[:].rearrange("(c x) n -> x c n", c=2), hb[:])
                    nc.gpsimd.collective_compute(
                        "AllGather", ALU.bypass,
                        replica_groups=[list(range(N_CORES))],
                        ins=[h_bounce[:]],
                        outs=[hT_full[:]],
                    )

                # ---- K/V table (all nodes) + Q table (own) ----
                with (
                    tc.tile_pool(name="kvp", bufs=4) as mp,
                    tc.tile_pool(name="kvps", bufs=3, space="PSUM") as pp,
                ):
                    for ch in range(NPAD // P):
                        blk, off = divmod(ch * P, NPCP)
                        ns = slice(ch * P, (ch + 1) * P)
                        hc = mp.tile([P, 2, P], FP, tag="hc")
                        nc.sync.dma_start(hc[:], hT_full[blk][:, off:off + P].rearrange("(c x) n -> x c n", c=2))
                        ps = pp.tile([P, 2 * HID], FP, tag="pkv", space="PSUM")
                        for ic in range(2):
                            nc.tensor.matmul(
                                out=ps[:], lhsT=hc[:, ic, :],
                                rhs=WkvT_s[:, ic, l, :],
                                start=(ic == 0), stop=((not qkv_bias) and ic == 1),
                            )
                        if qkv_bias:
                            nc.tensor.matmul(out=ps[:], lhsT=ones_row[:],
                                             rhs=bkv_s[:, l, :], start=False, stop=True)
                        kv = mp.tile([P, 2 * HID], FP, tag="kv")
                        nc.scalar.copy(out=kv[:], in_=ps[:])
                        nc.sync.dma_start(KVtab[ns, :], kv[:])
                    for ch in range(NPCP // P):
                        ns = slice(ch * P, (ch + 1) * P)
                        ps = pp.tile([P, HID], FP, tag="pq", space="PSUM")
                        for ic in range(2):
                            nc.tensor.matmul(
                                out=ps[:], lhsT=hn_sb[:, ic, ns],
                                rhs=WqT_s[:, ic, l, :],
                                start=(ic == 0), stop=((not qkv_bias) and ic == 1),
                            )
                        if qkv_bias:
                            nc.tensor.matmul(out=ps[:], lhsT=ones_row[:],
                                             rhs=bq_s[:, l, :], start=False, stop=True)
                        q = mp.tile([P, HID], FP, tag="q")
                        nc.scalar.copy(out=q[:], in_=ps[:])
                        nc.sync.dma_start(Qtab[ns, :], q[:])

                # ---- edge loop ----
                with (
                    tc.tile_pool(name="gath", bufs=4) as gp,
                    tc.tile_pool(name="work", bufs=2) as wp,
                    tc.tile_pool(name="small", bufs=4) as sp,
                    tc.tile_pool(name="eps", bufs=2, space="PSUM") as pp,
                ):
                    ps0 = ps1 = None
                    for t, (w, first, last) in enumerate(sched):
                        if first:
                            ps0 = pp.tile([P, P], FP, tag="ps0", space="PSUM")
                            ps1 = pp.tile([P, P], FP, tag="ps1", space="PSUM")
                        kvg = gp.tile([P, 2 * HID], FP, tag="kvg")
                        nc.gpsimd.indirect_dma_start(
                            out=kvg[:], out_offset=None, in_=KVtab[:],
                            in_offset=bass.IndirectOffsetOnAxis(ap=tgt_s[:, t:t + 1], axis=0))
                        qg = gp.tile([P, HID], FP, tag="qg")
                        nc.gpsimd.indirect_dma_start(
                            out=qg[:], out_offset=None, in_=Qtab[:],
                            in_offset=bass.IndirectOffsetOnAxis(ap=src_s[:, t:t + 1], axis=0))
                        Kv = kvg[:, 0:HID]
                        Vv = kvg[:, HID:2 * HID]

                        Pt = wp.tile([P, 4, 4, 64], FP, tag="Pt")
                        nc.vector.tensor_tensor(
                            out=Pt[:],
                            in0=qg[:].rearrange("p (h d) -> p h d", h=4).unsqueeze(2).broadcast_to([P, 4, 4, 64]),
                            in1=Kv.rearrange("p (g d) -> p g d", g=4).unsqueeze(1).broadcast_to([P, 4, 4, 64]),
                            op=ALU.mult)
                        S = sp.tile([P, 16], FP, tag="S")
                        nc.vector.reduce_sum(out=S[:], in_=Pt[:].rearrange("p h g d -> p (h g) d"), axis=AX.X)
                        E = sp.tile([P, 16], FP, tag="E")
                        nc.scalar.activation(out=E[:], in_=S[:], func=ACTF.Exp, scale=0.125)
                        D = sp.tile([P, 4], FP, tag="D")
                        nc.vector.reduce_sum(out=D[:], in_=E[:].rearrange("p (h g) -> p h g", h=4), axis=AX.X)
                        R = sp.tile([P, 4], FP, tag="R")
                        nc.vector.reciprocal(out=R[:], in_=D[:])
                        Wt = sp.tile([P, 4, 4], FP, tag="Wt")
                        nc.vector.tensor_tensor(out=Wt[:], in0=E[:].rearrange("p (h g) -> p h g", h=4),
                                                in1=R[:].unsqueeze(2).broadcast_to([P, 4, 4]), op=ALU.mult)
                        P2 = wp.tile([P, 4, 64, 4], FP, tag="P2")
                        nc.vector.tensor_tensor(
                            out=P2[:],
                            in0=Wt[:].unsqueeze(2).broadcast_to([P, 4, 64, 4]),
                            in1=Vv.rearrange("p (g d) -> p d g", g=4).unsqueeze(1).broadcast_to([P, 4, 64, 4]),
                            op=ALU.mult)
                        Seg = wp.tile([P, P], FP, tag="Seg")
                        nc.vector.tensor_scalar(out=Seg[:], in0=iota_s[:], scalar1=srcl_s[:, t:t + 1],
                                                scalar2=None, op0=ALU.is_equal)
                        for hc_i in range(2):
                            ps = ps0 if hc_i == 0 else ps1
                            for g in range(4):
                                nc.tensor.matmul(
                                    out=ps[:],
                                    lhsT=P2[:, 2 * hc_i:2 * hc_i + 2, :, g].rearrange("p a d -> p (a d)"),
                                    rhs=Seg[:],
                                    start=(first and g == 0), stop=False,
                                )
                        if last:
                            for hc_i in range(2):
                                ps = ps0 if hc_i == 0 else ps1
                                nc.tensor.matmul(
                                    out=ps[:],
                                    lhsT=Eemb_s[:, l, hc_i * P:(hc_i + 1) * P],
                                    rhs=TC_s[:, w * P:(w + 1) * P],
                                    start=False, stop=True,
                                )
                                nc.scalar.copy(out=Msb[:, hc_i, w * P:(w + 1) * P], in_=ps[:])

                # ---- Wo-proj + residual + LN + relu (own nodes) ----
                with (
                    tc.tile_pool(name="upd", bufs=3) as mp,
                    tc.tile_pool(name="upps", bufs=2, space="PSUM") as pp,
                    tc.tile_pool(name="upst", bufs=1, space="PSUM") as pp_st,
                    tc.tile_pool(name="upbc", bufs=1, space="PSUM") as pp_bc,
                ):
                    for ct in range(NCT):
                        cs = slice(ct * 512, (ct + 1) * 512)
                        for oc in range(2):
                            ps = pp.tile([P, 512], FP, tag="pm2", space="PSUM")
                            for ic in range(2):
                                nc.tensor.matmul(
                                    out=ps[:],
                                    lhsT=WoT_s[:, ic, l, oc * P:(oc + 1) * P],
                                    rhs=Msb[:, ic, cs],
                                    start=(ic == 0), stop=(ic == 1),
                                )
                            nc.vector.tensor_tensor(out=z_sb[:, oc, cs], in0=ps[:],
                                                    in1=hn_sb[:, oc, cs], op=ALU.add)
                            nc.vector.tensor_scalar(out=z_sb[:, oc, cs], in0=z_sb[:, oc, cs],
                                                    scalar1=bo_s[:, 2 * l + oc:2 * l + oc + 1],
                                                    scalar2=None, op0=ALU.add)
                        # stats over feature dim via ones-matmul
                        ps_sum = pp_st.tile([1, 512], FP, tag="pssum", space="PSUM")
                        ps_sq = pp_st.tile([1, 512], FP, tag="pssq", space="PSUM")
                        sq = mp.tile([P, 2, 512], FP, tag="sq")
                        for oc in range(2):
                            nc.scalar.activation(out=sq[:, oc, :], in_=z_sb[:, oc, cs], func=ACTF.Square)
                        for oc in range(2):
                            nc.tensor.matmul(out=ps_sum[:], lhsT=ones_col[:], rhs=z_sb[:, oc, cs],
                                             start=(oc == 0), stop=(oc == 1))
                        for oc in range(2):
                            nc.tensor.matmul(out=ps_sq[:], lhsT=ones_col[:], rhs=sq[:, oc, :],
                                             start=(oc == 0), stop=(oc == 1))
                        mu = mp.tile([1, 512], FP, tag="mu")
                        nc.scalar.activation(out=mu[:], in_=ps_sum[:], func=ACTF.Copy, scale=1.0 / HID)
                        var = mp.tile([1, 512], FP, tag="var")
                        nc.scalar.activation(out=var[:], in_=ps_sq[:], func=ACTF.Copy, scale=1.0 / HID)
                        musq = mp.tile([1, 512], FP, tag="musq")
                        nc.scalar.activation(out=musq[:], in_=mu[:], func=ACTF.Square)
                        nc.vector.tensor_tensor(out=var[:], in0=var[:], in1=musq[:], op=ALU.subtract)
                        lnv = mp.tile([1, 512], FP, tag="lnv")
                        nc.vector.tensor_scalar(out=lnv[:], in0=var[:], scalar1=float(EPS),
                                                scalar2=None, op0=ALU.add)
                        nc.scalar.activation(out=lnv[:], in_=lnv[:], func=ACTF.Ln)
                        rstd = mp.tile([1, 512], FP, tag="rstd")
                        nc.scalar.activation(out=rstd[:], in_=lnv[:], func=ACTF.Exp, scale=-0.5)
                        ps_mu = pp_bc.tile([P, 512], FP, tag="psmu", space="PSUM")
                        ps_rs = pp_bc.tile([P, 512], FP, tag="psrs", space="PSUM")
                        nc.tensor.matmul(out=ps_mu[:], lhsT=ones_row[:], rhs=mu[:], start=True, stop=True)
                        nc.tensor.matmul(out=ps_rs[:], lhsT=ones_row[:], rhs=rstd[:], start=True, stop=True)
                        for oc in range(2):
                            nc.vector.tensor_tensor(out=hn_sb[:, oc, cs], in0=z_sb[:, oc, cs],
                                                    in1=ps_mu[:], op=ALU.subtract)
                            nc.vector.tensor_tensor(out=hn_sb[:, oc, cs], in0=hn_sb[:, oc, cs],
                                                    in1=ps_rs[:], op=ALU.mult)
                            nc.vector.tensor_scalar(out=hn_sb[:, oc, cs], in0=hn_sb[:, oc, cs],
                                                    scalar1=lng_s[:, 2 * l + oc:2 * l + oc + 1],
                                                    scalar2=lnb_s[:, 2 * l + oc:2 * l + oc + 1],
                                                    op0=ALU.mult, op1=ALU.add)
                            nc.scalar.activation(out=hn_sb[:, oc, cs], in_=hn_sb[:, oc, cs], func=ACTF.Relu)

            # ---------------- FFN; adjustment out (residual added on host) ----------------
            with (
                tc.tile_pool(name="ffn", bufs=3) as mp,
                tc.tile_pool(name="ffps", bufs=2, space="PSUM") as pp,
            ):
                for ct in range(NCT):
                    cs = slice(ct * 512, (ct + 1) * 512)
                    for oc in range(2):
                        ps = pp.tile([P, 512], FP, tag="pf1", space="PSUM")
                        for ic in range(2):
                            nc.tensor.matmul(
                                out=ps[:], lhsT=W1T_s[:, ic, oc * P:(oc + 1) * P],
                                rhs=hn_sb[:, ic, cs],
                                start=(ic == 0), stop=(ic == 1),
                            )
                        nc.scalar.activation(out=f1_sb[:, oc, cs], in_=ps[:], func=ACTF.Relu,
                                             bias=bias_s[:, 2 + oc:3 + oc], scale=1.0)
                # f2 (the adjustment) lands in hn_sb (free by now), then is
                # quantized to int8 with a per-feature scale computed on-device
                for ct in range(NCT):
                    cs = slice(ct * 512, (ct + 1) * 512)
                    for oc in range(2):
                        ps = pp.tile([P, 512], FP, tag="pf2", space="PSUM")
                        for ic in range(2):
                            nc.tensor.matmul(
                                out=ps[:], lhsT=W2T_s[:, ic, oc * P:(oc + 1) * P],
                                rhs=f1_sb[:, ic, cs],
                                start=(ic == 0), stop=(ic == 1),
                            )
                        nc.scalar.activation(out=hn_sb[:, oc, cs], in_=ps[:], func=ACTF.Identity,
                                             bias=bias_s[:, 4 + oc:5 + oc], scale=1.0)
                amax = mp.tile([P, 2], FP, tag="amax")
                rec = mp.tile([P, 2], FP, tag="rec")
                scl2 = mp.tile([P, 2], FP, tag="scl2")
                q8 = mp.tile([P, 2, NPCP], I8, tag="q8")
                for oc in range(2):
                    nc.vector.reduce_max(out=amax[:, oc:oc + 1], in_=hn_sb[:, oc, :],
                                         axis=AX.X, apply_absolute_value=True)
                nc.vector.tensor_scalar(out=amax[:], in0=amax[:], scalar1=1e-20,
                                        scalar2=None, op0=ALU.max)
                nc.vector.reciprocal(out=rec[:], in_=amax[:])
                nc.vector.tensor_scalar(out=scl2[:], in0=amax[:], scalar1=1.0 / 127.0,
                                        scalar2=None, op0=ALU.mult)
                for oc in range(2):
                    nc.vector.tensor_scalar(out=q8[:, oc, :], in0=hn_sb[:, oc, :],
                                            scalar1=rec[:, oc:oc + 1], scalar2=127.0,
                                            op0=ALU.mult, op1=ALU.mult)
                nc.sync.dma_start(out_i8[:, 0:NPCP].rearrange("(c x) n -> x c n", c=2), q8[:])
                nc.sync.dma_start(
                    out_i8[:, NPCP:NPCP + 4].rearrange("(c x) n -> x c n", c=2),
                    scl2[:].bitcast(I8).rearrange("p (c b) -> p c b", c=2))
                nc.gpsimd.collective_compute(
                    "AllGather", ALU.bypass,
                    replica_groups=[list(range(N_CORES))],
                    ins=[out_i8[:]],
                    outs=[out_gath[:]],
                )
                nc.sync.dma_start(out_all[:], out_gath[:])

    nc.compile()
    return nc


def _chunk_wT(W):
    """W [O, I] or [L, O, I] -> device layout [128, 2, (L,) O] where
    arr[x, c, (l,) o] = W[(l,) o, c*128+x]."""
    W = np.asarray(W, np.float32)
    if W.ndim == 2:
        A = W.T.reshape(2, P, W.shape[0])            # [c, x, o]
        return np.ascontiguousarray(A.transpose(1, 0, 2))
    A = W.transpose(2, 0, 1).reshape(2, P, W.shape[0], W.shape[1])  # [c, x, l, o]
    return np.ascontiguousarray(A.transpose(1, 0, 2, 3))


def _pack2(*vs):
    # each v [256] -> [128, 2]; concat on cols
    cols = []
    for v in vs:
        cols.append(np.asarray(v, np.float32).reshape(2, P).T)
    return np.ascontiguousarray(np.concatenate(cols, axis=1))


def kernel(x, edges, edge_types, Win, b_in, Wq, bq, Wk, bk, Wv, bv,
           Eemb, Wo, bo, ln_g, ln_b, W1, b1, W2, b2):
    x = np.asarray(x, np.float32)
    NTIL, sched, tgt_ix, src_ix, srcl_f, TC = _prep_edges_cached(
        np.asarray(edges), np.asarray(edge_types))

    qkv_bias = bool(np.any(np.asarray(bq)) or np.any(np.asarray(bk)) or np.any(np.asarray(bv)))
    key = (NTIL, tuple(w for w, _, _ in sched), qkv_bias)
    if key not in _CACHE:
        _CACHE[key] = _build_program(NTIL, sched, qkv_bias)
    nc = _CACHE[key]

    # per-feature int8 quantization of x; the dequant scale ships in the blob
    # (quantize against the fp16-rounded scale the device will actually use)
    sx = np.maximum(np.abs(x).max(axis=0) / 127.0, 1e-20)
    sx = sx.astype(np.float16).astype(np.float32)
    y = x * (1.0 / sx)
    np.rint(y, out=y)
    np.clip(y, -127, 127, out=y)
    x8T = np.ascontiguousarray(y.astype(np.int8).T)  # [HID, N_NODES]

    parts = {
        "bias_pk": _pack2(b_in, b1, b2),
        "lng_pk": _pack2(*[np.asarray(ln_g, np.float32)[l] for l in range(N_LAYERS)]),
        "lnb_pk": _pack2(*[np.asarray(ln_b, np.float32)[l] for l in range(N_LAYERS)]),
        "bo_pk": _pack2(*[np.asarray(bo, np.float32)[l] for l in range(N_LAYERS)]),
        "bkv": np.concatenate([np.asarray(bk, np.float32), np.asarray(bv, np.float32)],
                              axis=1).reshape(1, N_LAYERS, 2 * HID),
        "bq": np.asarray(bq, np.float32).reshape(1, N_LAYERS, HID),
        "Eemb": np.ascontiguousarray(np.transpose(np.asarray(Eemb, np.float32), (1, 0, 2))),
        "iota": np.ascontiguousarray(np.broadcast_to(np.arange(P, dtype=np.float32), (P, P))),
        "sx_pk": np.ascontiguousarray(sx.reshape(2, P).T),
        "wbase": np.asarray([[float(w * P) for w, _, _ in sched]], np.float32),
    }
    for name, Wt in [
        ("WinT", _chunk_wT(Win)),
        ("WqT3", _chunk_wT(Wq)),
        ("WkvT3", np.concatenate([_chunk_wT(Wk), _chunk_wT(Wv)], axis=3)),
        ("WoT3", _chunk_wT(Wo)),
        ("W1T", _chunk_wT(W1)),
        ("W2T", _chunk_wT(W2)),
    ]:
        # int8 quantize per input channel (the partition/chunk axes of the
        # transposed layout); scale ships fp16 and is applied on-device
        s = np.maximum(np.abs(Wt).max(axis=tuple(range(2, Wt.ndim))) / 127.0, 1e-20)
        s = s.astype(np.float16).astype(np.float32)
        q = np.clip(np.rint(Wt / s[(...,) + (None,) * (Wt.ndim - 2)]), -127, 127)
        parts[name] = q.astype(np.int8)
        parts[name + "$s"] = s.astype(np.float16)

    BLOB_OFF, BLOB_TOT = _blob_layout(NTIL)
    BLOB_CHUNK = BLOB_TOT // N_CORES
    blob = np.zeros(BLOB_TOT, np.int8)
    for name, (o, s, dt) in BLOB_OFF.items():
        a = parts[name]
        assert a.shape == s, (name, a.shape, s)
        if dt == F16 and a.dtype != np.float16:
            a = a.astype(np.float16)
        blob[o:o + a.nbytes] = a.ravel().view(np.int8)

    MEGA, MEGA_NB = _mega_layout(NTIL)

    def fill(m, name, arr):
        o, nb = MEGA[name]
        m[o:o + nb] = arr.ravel().view(np.int8)

    in_maps = []
    for c in range(N_CORES):
        xT8 = np.zeros((HID, NPCP), np.int8)
        xT8[:, :NPC] = x8T[:, c * NPC:(c + 1) * NPC]
        m = np.zeros(MEGA_NB, np.int8)
        fill(m, "x8", xT8)
        fill(m, "TC", TC[c])
        fill(m, "wchunk", blob[c * BLOB_CHUNK:(c + 1) * BLOB_CHUNK])
        fill(m, "tgt16", tgt_ix[c])
        fill(m, "srcl8", srcl_f[c])
        in_maps.append({"mega": m})

    try:
        G = _get_runner(nc)(in_maps)["out_all"]       # [N_CORES, HID, NPCP+4]
    except Exception as e:
        import traceback
        print(f"cached runner failed ({e!r}); falling back to run_bass_kernel_spmd")
        traceback.print_exc()
        G = run_bass_kernel_spmd(nc, in_maps, list(range(N_CORES))).results[0]["out_all"]
    out = np.empty((N_NODES, HID), np.float32)
    for c in range(N_CORES):
        buf = G[c]                                    # [HID, NPCP+4] int8
        scl = buf[:, NPCP:NPCP + 4].copy().view(np.float32).ravel()  # [HID]
        sl = slice(c * NPC, (c + 1) * NPC)
        np.multiply(buf[:, :NPC].T, scl[None, :], out=out[sl])
        out[sl] += x[sl]
    return out
